# revision 1
# baseline (speedup 1.0000x reference)
"""Trainium2 Bass kernel for nn_DegreePrediction.

Computes y[u] = sum_{s,t,v} (x*W_t)[s,t] * (W_r*r_zeros + r_const)[s,t,u,v]
with N=80, streaming the three rank-4 tensors from HBM.

Sharding: leading s axis split across 8 cores (10 s-values = 800 (s,t) rows
per core, contiguous in DRAM). Each core computes a partial y[80]; partials
are summed on the host (the output is tiny, so no device collective).

The kernel is purely HBM/DMA-bound (arithmetic intensity ~0.33 flop/byte);
the big tensors are shipped as fp16 (halves DMA traffic; all arithmetic and
accumulation stay fp32 — absmax error ~1e-3 of output scale).

Per-core device schedule (7 blocks of K<=128 (s,t)-rows):
  DMA   : wr/rz/rc block tiles [K, 80, 80] fp16 (12.8KB contiguous per row)
  DVE   : comb(f32) = wr*rz ; comb += rc ; c2 = reduce_v(comb) -> [K, 80]
  PE    : psum_y[1,80] += layer2_block[K,1].T @ c2[K,80]  (PSUM-accumulated)
"""

import numpy as np

import concourse.bacc as bacc
import concourse.mybir as mybir
import concourse.tile as tile
from concourse.bass_utils import run_bass_kernel_spmd

N = 80
N_CORES = 8
S_PER_CORE = N // N_CORES            # 10
ST = S_PER_CORE * N                  # 800 (s,t) rows per core
N_BLOCKS = 7                         # 6*128 + 32
F32 = mybir.dt.float32
F16 = mybir.dt.float16

_CACHE = {}


def build_nc(repeats=1):
    nc = bacc.Bacc()
    wr_d = nc.declare_dram_parameter("wr", [ST, N, N], F16, isOutput=False)
    rz_d = nc.declare_dram_parameter("rz", [ST, N, N], F16, isOutput=False)
    rc_d = nc.declare_dram_parameter("rc", [ST, N, N], F16, isOutput=False)
    l2_d = nc.declare_dram_parameter("l2", [128, N_BLOCKS], F32, isOutput=False)
    y_d = nc.declare_dram_parameter("y", [1, N], F32, isOutput=True)

    with tile.TileContext(nc) as tc:
        with (
            tc.tile_pool(name="io", bufs=2) as pool,
            tc.tile_pool(name="small", bufs=1) as sp,
            tc.psum_pool(name="ps", bufs=1) as pp,
        ):
            l2_sb = sp.tile([128, N_BLOCKS], F32)
            nc.sync.dma_start(out=l2_sb[:], in_=l2_d[:])
            ypsum = pp.tile([1, N], F32)

            for r in range(repeats):
                for b in range(N_BLOCKS):
                    r0 = b * 128
                    K = min(128, ST - r0)
                    wr_t = pool.tile([128, N, N], F16, tag="wr", bufs=3)
                    rz_t = pool.tile([128, N, N], F16, tag="rz", bufs=3)
                    rc_t = pool.tile([128, N, N], F16, tag="rc", bufs=3)
                    nc.sync.dma_start(out=wr_t[:K], in_=wr_d[r0 : r0 + K])
                    nc.scalar.dma_start(out=rz_t[:K], in_=rz_d[r0 : r0 + K])
                    nc.sync.dma_start(out=rc_t[:K], in_=rc_d[r0 : r0 + K])

                    comb = pool.tile([128, N, N], F32, tag="comb")
                    nc.vector.tensor_mul(out=comb[:K], in0=wr_t[:K], in1=rz_t[:K])
                    nc.vector.tensor_add(out=comb[:K], in0=comb[:K], in1=rc_t[:K])

                    c2 = pool.tile([128, N], F32, tag="c2")
                    nc.vector.tensor_reduce(
                        out=c2[:K],
                        in_=comb[:K],
                        axis=mybir.AxisListType.X,
                        op=mybir.AluOpType.add,
                    )
                    nc.tensor.matmul(
                        ypsum[:],
                        l2_sb[0:K, b : b + 1],
                        c2[:K],
                        start=(b == 0),
                        stop=(b == N_BLOCKS - 1),
                    )

            y_sb = sp.tile([1, N], F32)
            nc.vector.tensor_copy(out=y_sb[:], in_=ypsum[:])
            nc.sync.dma_start(out=y_d[:], in_=y_sb[:])
    nc.compile()
    return nc


def _get_nc():
    if "nc" not in _CACHE:
        _CACHE["nc"] = build_nc()
    return _CACHE["nc"]


def make_in_maps(x, r_zeros, r_const, weights_t, weights_r):
    l2 = (np.asarray(x, np.float32) * np.asarray(weights_t, np.float32))
    wr16 = np.asarray(weights_r, np.float32).astype(np.float16)
    rz16 = np.asarray(r_zeros, np.float32).astype(np.float16)
    rc16 = np.asarray(r_const, np.float32).astype(np.float16)
    in_maps = []
    for c in range(N_CORES):
        sl = slice(c * S_PER_CORE, (c + 1) * S_PER_CORE)
        l2p = np.zeros(128 * N_BLOCKS, np.float32)
        l2p[:ST] = l2[sl].reshape(-1)
        l2cols = np.ascontiguousarray(l2p.reshape(N_BLOCKS, 128).T)
        in_maps.append(
            {
                "wr": wr16[sl].reshape(ST, N, N),
                "rz": rz16[sl].reshape(ST, N, N),
                "rc": rc16[sl].reshape(ST, N, N),
                "l2": l2cols,
            }
        )
    return in_maps


def run(x, r_zeros, r_const, weights_t, weights_r, **spmd_kwargs):
    nc = _get_nc()
    in_maps = make_in_maps(x, r_zeros, r_const, weights_t, weights_r)
    res = run_bass_kernel_spmd(nc, in_maps, list(range(N_CORES)), **spmd_kwargs)
    y = np.zeros(N, np.float32)
    for i in range(N_CORES):
        y += res.results[i]["y"].reshape(N)
    return y, res


def kernel(x, r_zeros, r_const, weights_t, weights_r):
    y, _ = run(x, r_zeros, r_const, weights_t, weights_r)
    return y



# revision 15
# speedup vs baseline: 1.5321x; 1.5321x over previous
"""Trainium2 Bass kernel for nn_DegreePrediction.

Computes y[u] = sum_{s,t,v} (x*W_t)[s,t] * (W_r*r_zeros + r_const)[s,t,u,v]
with N=80, streaming the three rank-4 tensors from HBM as fp16.

Sharding: leading s axis split across 8 cores (10 s-values = 800 (s,t) rows
per core, contiguous in DRAM). Each core computes partial outputs; partials
are summed on the host (the output is tiny, so no device collective).

The computation is split into two streams so every engine stays under the
DMA roofline (~30.7MB/core fp16 @ ~350GB/s):

  stream1 (wr*rz):  DVE  comb16 = wr*rz (fp16)
                    DVE  c1[K,80] = reduce_v(comb16)  (f32)
                    PE   ypsum[1,80] += l2_f32[K,1].T @ c1[K,80]
  stream2 (rc):     PE   psum2[2,40,80] += l2_f16[K,1].T @ rc[K,chunk]
                    (rc never touches the vector engine at all; the PE
                     contracts over (s,t) in 320..480-col chunks and the
                     v-reduction of the [80,80] accumulator happens once
                     at the end)

Per-core totals: DVE ~2 passes/block (was 3 f32 passes = 143us), PE ~6400
cols/block of K<=128 matmul, DMA unchanged.
"""

import numpy as np

import concourse.bacc as bacc
import concourse.mybir as mybir
import concourse.tile as tile
from concourse.bass_utils import run_bass_kernel_spmd

N = 80
N_CORES = 8
S_PER_CORE = N // N_CORES            # 10
ST = S_PER_CORE * N                  # 800 (s,t) rows per core
N_BLOCKS = 7                         # 6*128 + 32
F32 = mybir.dt.float32
F16 = mybir.dt.float16

# stream2 PSUM layout: 3 accumulator rows on partitions 0/32/64 (the legal
# PSUM AP bases), covering u 0:27 / 27:54 / 54:80, each <= 2160 f32 (8.6KB of
# the 16KB per-partition PSUM). Chunks of <=6 u-groups keep each matmul under
# the 512 moving-column limit.
ROW_U = 27                           # max u values per psum row
ROWS = [(0, 0, 27), (32, 27, 27), (64, 54, 26)]  # (partition, u0, u_count)


def _chunks(count):
    return [(s, min(6, count - s)) for s in range(0, count, 6)]

_CACHE = {}


def build_nc(repeats=1):
    nc = bacc.Bacc()
    wr_d = nc.declare_dram_parameter("wr", [ST, N, N], F16, isOutput=False)
    rz_d = nc.declare_dram_parameter("rz", [ST, N, N], F16, isOutput=False)
    rc_d = nc.declare_dram_parameter("rc", [ST, N * N], F16, isOutput=False)
    l2f_d = nc.declare_dram_parameter("l2f", [128, N_BLOCKS], F32, isOutput=False)
    l2h_d = nc.declare_dram_parameter("l2h", [128, N_BLOCKS], F16, isOutput=False)
    y1_d = nc.declare_dram_parameter("y1", [1, N], F32, isOutput=True)
    y2_d = nc.declare_dram_parameter("y2", [len(ROWS), ROW_U], F32, isOutput=True)

    with tile.TileContext(nc) as tc:
        with (
            tc.tile_pool(name="io", bufs=2) as pool,
            tc.tile_pool(name="small", bufs=1) as sp,
            tc.psum_pool(name="ps", bufs=1) as pp,
        ):
            l2f_sb = sp.tile([128, N_BLOCKS], F32)
            l2h_sb = sp.tile([128, N_BLOCKS], F16)
            nc.sync.dma_start(out=l2f_sb[:], in_=l2f_d[:])
            nc.sync.dma_start(out=l2h_sb[:], in_=l2h_d[:])
            ypsum = pp.tile([1, N], F32)
            psum2 = pp.tile([65, ROW_U, N], F32)
            # zero the stream2 accumulator once and accumulate with
            # start=False throughout: per-matmul start=True resets at PSUM
            # bank granularity, which clobbers neighbouring chunks that
            # share a bank.
            nc.vector.memset(psum2[:], 0.0)

            for r in range(repeats):
                for b in range(N_BLOCKS):
                    r0 = b * 128
                    K = min(128, ST - r0)
                    wr_t = pool.tile([128, N, N], F16, tag="wr", bufs=3)
                    rz_t = pool.tile([128, N, N], F16, tag="rz", bufs=3)
                    rc_t = pool.tile([128, N * N], F16, tag="rc", bufs=3)
                    nc.sync.dma_start(out=wr_t[:K], in_=wr_d[r0 : r0 + K])
                    nc.scalar.dma_start(out=rz_t[:K], in_=rz_d[r0 : r0 + K])
                    nc.sync.dma_start(out=rc_t[:K], in_=rc_d[r0 : r0 + K])

                    start = b == 0
                    stop = b == N_BLOCKS - 1

                    # stream2: PE eats rc directly, contraction over (s,t)
                    for p, ubase, ucount in ROWS:
                        for u0, un in _chunks(ucount):
                            c0 = (ubase + u0) * N
                            nc.tensor.matmul(
                                psum2[p : p + 1, u0 : u0 + un, :],
                                l2h_sb[0:K, b : b + 1],
                                rc_t[:K, c0 : c0 + un * N],
                                start=False,
                                stop=stop,
                                skip_group_check=True,
                            )

                    # stream1: DVE mul + v-reduce, then a tiny matmul
                    comb16 = pool.tile([128, N, N], F16, tag="comb")
                    nc.vector.tensor_mul(out=comb16[:K], in0=wr_t[:K], in1=rz_t[:K])
                    c1 = pool.tile([128, N], F32, tag="c1")
                    nc.vector.tensor_reduce(
                        out=c1[:K],
                        in_=comb16[:K],
                        axis=mybir.AxisListType.X,
                        op=mybir.AluOpType.add,
                    )
                    nc.tensor.matmul(
                        ypsum[:],
                        l2f_sb[0:K, b : b + 1],
                        c1[:K],
                        start=start,
                        stop=stop,
                    )

            y1_sb = sp.tile([1, N], F32)
            nc.vector.tensor_copy(out=y1_sb[:], in_=ypsum[:])
            nc.sync.dma_start(out=y1_d[:], in_=y1_sb[:])
            y2_sb = sp.tile([65, ROW_U], F32)
            for row, (p, ubase, ucount) in enumerate(ROWS):
                nc.vector.tensor_reduce(
                    out=y2_sb[p : p + 1, 0:ucount],
                    in_=psum2[p : p + 1, 0:ucount, :],
                    axis=mybir.AxisListType.X,
                    op=mybir.AluOpType.add,
                )
                nc.sync.dma_start(
                    out=y2_d[row : row + 1, 0:ucount], in_=y2_sb[p : p + 1, 0:ucount]
                )
    nc.compile()
    return nc


def _get_nc():
    if "nc" not in _CACHE:
        _CACHE["nc"] = build_nc()
    return _CACHE["nc"]


def make_in_maps(x, r_zeros, r_const, weights_t, weights_r):
    l2 = np.asarray(x, np.float32) * np.asarray(weights_t, np.float32)
    wr16 = np.asarray(weights_r, np.float32).astype(np.float16)
    rz16 = np.asarray(r_zeros, np.float32).astype(np.float16)
    rc16 = np.asarray(r_const, np.float32).astype(np.float16)
    in_maps = []
    for c in range(N_CORES):
        sl = slice(c * S_PER_CORE, (c + 1) * S_PER_CORE)
        l2p = np.zeros(128 * N_BLOCKS, np.float32)
        l2p[:ST] = l2[sl].reshape(-1)
        l2cols = np.ascontiguousarray(l2p.reshape(N_BLOCKS, 128).T)
        in_maps.append(
            {
                "wr": wr16[sl].reshape(ST, N, N),
                "rz": rz16[sl].reshape(ST, N, N),
                "rc": rc16[sl].reshape(ST, N * N),
                "l2f": l2cols,
                "l2h": l2cols.astype(np.float16),
            }
        )
    return in_maps


def run(x, r_zeros, r_const, weights_t, weights_r, **spmd_kwargs):
    nc = _get_nc()
    in_maps = make_in_maps(x, r_zeros, r_const, weights_t, weights_r)
    res = run_bass_kernel_spmd(nc, in_maps, list(range(N_CORES)), **spmd_kwargs)
    y = np.zeros(N, np.float32)
    for i in range(N_CORES):
        y += res.results[i]["y1"].reshape(N)
        y2 = res.results[i]["y2"]
        for row, (_, ubase, ucount) in enumerate(ROWS):
            y[ubase : ubase + ucount] += y2[row, 0:ucount]
    return y, res


def kernel(x, r_zeros, r_const, weights_t, weights_r):
    y, _ = run(x, r_zeros, r_const, weights_t, weights_r)
    return y
